# revision 5
# baseline (speedup 1.0000x reference)
"""CQAttention (BiDAF context-query attention) forward kernel for 8 Trainium2
NeuronCores.

Full inputs: context (64,128,1024) f32, question (64,128,128) f32, w (384,) f32.
Full output: (64, 512, 1024) f32.

Sharding: pure data parallel over batch — 8 batches per core, w replicated.

Math (per batch, X = context[b] (H,C), Y = question[b] (H,Q), w=(wq,wc,wcq)):
    S^T = (wcq*Y + wc 1^T)^T @ X              # (Q,C); wq term is softmax-invariant
    P   = exp(S^T)                            # unnormalized softmax numerators
    d   = rowsum(P); r = 1/d                  # softmax denominators (per q-row)
    A   = (diag(r) Y^T)^T @ P                 # = a^T                (H,C)
    tt  = P @ X^T                             # (Q,H) via PE transposes of P,X
    Bm  = (diag(r^2) tt)^T @ P                # = b^T = (s1 (s1^T c))^T  (H,C)
    out = [X; A; X*A; X*Bm]                   # (4H, C)

All matmuls run in float32r (TF32, 1 cycle/row at N>=256 vs 4 for fp32);
rounding to f32r happens inside ops that already exist (cast-on-write).
"""

import os
import sys

import numpy as np

if "/opt/trn_rl_repo" not in sys.path:
    sys.path.insert(0, "/opt/trn_rl_repo")

B, H, C, Q = 64, 128, 1024, 128
NCORES = 8
BPC = B // NCORES  # batches per core


def _ensure_ntff_hook():
    """This container's `antenv` stub lacks `axon_hooks`, which
    bass_utils needs for NTFF profiling under axon (trace=True). Install
    a functional shadow module + register the ctypes-based hook."""
    import types

    try:
        from antenv.axon_hooks import get_axon_ntff_profile_hook  # noqa: F401

        return  # real module present
    except ImportError:
        pass
    try:
        import antenv

        mod = types.ModuleType("antenv.axon_hooks")
        _state = {"hook": None}

        def set_axon_ntff_profile_hook(h):
            _state["hook"] = h

        def get_axon_ntff_profile_hook():
            return _state["hook"]

        mod.set_axon_ntff_profile_hook = set_axon_ntff_profile_hook
        mod.get_axon_ntff_profile_hook = get_axon_ntff_profile_hook
        sys.modules["antenv.axon_hooks"] = mod
        antenv.axon_hooks = mod

        from trn_agent_boot.trn_boot import _ntff_profile_via_ctypes

        set_axon_ntff_profile_hook(
            _ntff_profile_via_ctypes("/opt/axon/libaxon_pjrt.so")
        )
    except Exception:
        pass  # profiling degrades; compute still works


_ensure_ntff_hook()

LAST_RESULTS = None
_NC = None


def _build():
    from contextlib import ExitStack

    import concourse.bacc as bacc
    import concourse.mybir as mybir
    import concourse.tile as tile
    from concourse import masks

    f32 = mybir.dt.float32
    f32r = mybir.dt.float32r
    MULT = mybir.AluOpType.mult
    ADD = mybir.AluOpType.add
    EXP = mybir.ActivationFunctionType.Exp

    nc = bacc.Bacc(
        "TRN2", target_bir_lowering=False, debug=False, enable_asserts=False
    )
    ctx_t = nc.dram_tensor("context", (BPC, H, C), f32, kind="ExternalInput").ap()
    q_t = nc.dram_tensor("question", (BPC, H, Q), f32, kind="ExternalInput").ap()
    w_t = nc.dram_tensor("w", (3 * H,), f32, kind="ExternalInput").ap()
    out_t = nc.dram_tensor("out", (BPC, 4 * H, C), f32, kind="ExternalOutput").ap()

    with tile.TileContext(nc) as tc, ExitStack() as ctx:
        const = ctx.enter_context(tc.tile_pool(name="const", bufs=1))
        sb = ctx.enter_context(tc.tile_pool(name="sb", bufs=2))
        ps_score = ctx.enter_context(tc.tile_pool(name="pscore", bufs=2, space="PSUM"))
        ps_tr = ctx.enter_context(tc.tile_pool(name="ptr", bufs=3, space="PSUM"))
        ps_tt = ctx.enter_context(tc.tile_pool(name="ptt", bufs=1, space="PSUM"))

        ident = const.tile([128, 128], f32, tag="ident")
        masks.make_identity(nc, ident[:])
        identr = const.tile([128, 128], f32r, tag="identr")
        nc.vector.tensor_copy(identr[:], ident[:])

        wc = const.tile([128, 1], f32, tag="wc")
        wcq = const.tile([128, 1], f32, tag="wcq")
        nc.sync.dma_start(wc[:], w_t[H : 2 * H].unsqueeze(1))
        nc.sync.dma_start(wcq[:], w_t[2 * H : 3 * H].unsqueeze(1))

        for b in range(BPC):
            X = sb.tile([H, C], f32, tag="X")
            nc.sync.dma_start(X[:], ctx_t[b])
            Y = sb.tile([H, Q], f32, tag="Y")
            nc.sync.dma_start(Y[:], q_t[b])

            # f32r-rounded copies for matmul inputs
            Xr = sb.tile([H, C], f32r, tag="Xr")
            nc.vector.tensor_copy(Xr[:], X[:])
            Yr = sb.tile([H, Q], f32r, tag="Yr")
            nc.vector.tensor_copy(Yr[:], Y[:])

            # Z = wcq * Y + wc  (so Z^T @ X = G + 1 cw^T, the softmax logits)
            Z = sb.tile([H, Q], f32r, tag="Z")
            nc.vector.tensor_scalar(Z[:], Y[:], wcq[:], wc[:], op0=MULT, op1=ADD)

            S = ps_score.tile([Q, C], f32, tag="score")
            for j in range(0, C, 512):
                nc.tensor.matmul(
                    S[:, j : j + 512],
                    Z[:],
                    Xr[:, j : j + 512],
                    start=True,
                    stop=True,
                )

            # P = exp(S), d = rowsum(P)  (no max-subtraction: |S| <~ 10)
            P = sb.tile([Q, C], f32r, tag="P")
            dsum = sb.tile([Q, 1], f32, tag="dsum")
            nc.scalar.activation(P[:], S[:], EXP, accum_out=dsum[:])
            rr = sb.tile([Q, 1], f32, tag="rr")
            nc.vector.reciprocal(rr[:], dsum[:])
            r2 = sb.tile([Q, 1], f32, tag="r2")
            nc.vector.tensor_mul(r2[:], rr[:], rr[:])

            # XT holds [YTs | X^T chunks 0..7]; the leading YTs block means
            # every N=256 tt-matmul window reads initialized data.
            XT = sb.tile([128, 128 + C], f32r, tag="XT")

            # YTs = diag(r) Y^T  -> XT[:, 0:128]
            yt = ps_tr.tile([128, 512], f32, tag="tr")
            nc.tensor.transpose(yt[:, 0:128].bitcast(f32r), Yr[:], identr[:])
            YTs = XT[:, 0:128]
            nc.vector.tensor_scalar_mul(YTs, yt[:, 0:128], rr[:])

            # P^T and X^T chunks (PE transposes; copies to SBUF on ScalarE)
            PT = sb.tile([128, C], f32r, tag="PT")
            for g in range(2):
                ptp = ps_tr.tile([128, 512], f32, tag="tr")
                for k in range(4):
                    c0 = g * 4 + k
                    nc.tensor.transpose(
                        ptp[:, k * 128 : (k + 1) * 128].bitcast(f32r),
                        P[:, c0 * 128 : (c0 + 1) * 128],
                        identr[:],
                    )
                nc.scalar.copy(PT[:, g * 512 : (g + 1) * 512], ptp[:])

            for g in range(2):
                xtp = ps_tr.tile([128, 512], f32, tag="tr")
                for k in range(4):
                    c0 = g * 4 + k
                    nc.tensor.transpose(
                        xtp[:, k * 128 : (k + 1) * 128].bitcast(f32r),
                        Xr[:, c0 * 128 : (c0 + 1) * 128],
                        identr[:],
                    )
                nc.scalar.copy(XT[:, 128 + g * 512 : 128 + (g + 1) * 512], xtp[:])

            # tt[:,128:256] = P @ X^T  (cols 0:128 accumulate junk, never read)
            tt = ps_tt.tile([Q, 256], f32, tag="tt")
            for c in range(8):
                nc.tensor.matmul(
                    tt[:],
                    PT[:, c * 128 : (c + 1) * 128],
                    XT[:, c * 128 : c * 128 + 256],
                    start=(c == 0),
                    stop=(c == 7),
                )
            tts = sb.tile([Q, H], f32r, tag="tts")
            nc.vector.tensor_scalar_mul(tts[:], tt[:, 128:256], r2[:])

            Aps = ps_score.tile([H, C], f32, tag="score")
            for j in range(0, C, 512):
                nc.tensor.matmul(
                    Aps[:, j : j + 512],
                    YTs,
                    P[:, j : j + 512],
                    start=True,
                    stop=True,
                )
            A = sb.tile([H, C], f32, tag="A")
            nc.scalar.copy(A[:], Aps[:])

            Bps = ps_score.tile([H, C], f32, tag="score")
            for j in range(0, C, 512):
                nc.tensor.matmul(
                    Bps[:, j : j + 512],
                    tts[:],
                    P[:, j : j + 512],
                    start=True,
                    stop=True,
                )
            Bsb = sb.tile([H, C], f32, tag="B")
            nc.scalar.copy(Bsb[:], Bps[:])

            XA = sb.tile([H, C], f32, tag="XA")
            nc.vector.tensor_mul(XA[:], X[:], A[:])
            XB = sb.tile([H, C], f32, tag="XB")
            nc.vector.tensor_mul(XB[:], X[:], Bsb[:])

            nc.sync.dma_start(out_t[b, 0:H], X[:])
            nc.sync.dma_start(out_t[b, H : 2 * H], A[:])
            nc.sync.dma_start(out_t[b, 2 * H : 3 * H], XA[:])
            nc.sync.dma_start(out_t[b, 3 * H : 4 * H], XB[:])

    nc.compile()
    return nc


def kernel(context, question, w):
    global _NC, LAST_RESULTS
    from concourse import bass_utils

    if _NC is None:
        _NC = _build()

    context = np.ascontiguousarray(np.asarray(context), dtype=np.float32)
    question = np.ascontiguousarray(np.asarray(question), dtype=np.float32)
    w = np.ascontiguousarray(np.asarray(w), dtype=np.float32)

    in_maps = [
        {
            "context": context[c * BPC : (c + 1) * BPC],
            "question": question[c * BPC : (c + 1) * BPC],
            "w": w,
        }
        for c in range(NCORES)
    ]
    trace = bool(int(os.environ.get("KTRACE", "0")))
    LAST_RESULTS = bass_utils.run_bass_kernel_spmd(
        _NC, in_maps, core_ids=list(range(NCORES)), trace=trace
    )
    out = np.concatenate(
        [LAST_RESULTS.results[c]["out"] for c in range(NCORES)], axis=0
    )
    return out


# revision 8
# speedup vs baseline: 1.1210x; 1.1210x over previous
"""CQAttention (BiDAF context-query attention) forward kernel for 8 Trainium2
NeuronCores.

Full inputs: context (64,128,1024) f32, question (64,128,128) f32, w (384,) f32.
Full output: (64, 512, 1024) f32.

Sharding: pure data parallel over batch — 8 batches per core, w replicated.

Math (per batch, X = context[b] (H,C), Y = question[b] (H,Q), w=(wq,wc,wcq)):
    S^T = (wcq*Y + wc 1^T)^T @ X              # (Q,C); wq term is softmax-invariant
    P   = exp(S^T)                            # unnormalized softmax numerators
    d   = rowsum(P); r = 1/d                  # softmax denominators (per q-row)
    A   = (diag(r) Y^T)^T @ P                 # = a^T                (H,C)
    tt  = P @ X^T                             # (Q,H) via PE transposes of P,X
    Bm  = (diag(r^2) tt)^T @ P                # = b^T = (s1 (s1^T c))^T  (H,C)
    out = [X; A; X*A; X*Bm]                   # (4H, C)

All matmuls run in float32r (TF32, 1 cycle/row at N>=256 vs 4 for fp32);
rounding to f32r happens inside ops that already exist (cast-on-write).
"""

import os
import sys

import numpy as np

if "/opt/trn_rl_repo" not in sys.path:
    sys.path.insert(0, "/opt/trn_rl_repo")

B, H, C, Q = 64, 128, 1024, 128
NCORES = 8
BPC = B // NCORES  # batches per core


def _ensure_ntff_hook():
    """This container's `antenv` stub lacks `axon_hooks`, which
    bass_utils needs for NTFF profiling under axon (trace=True). Install
    a functional shadow module + register the ctypes-based hook."""
    import types

    try:
        from antenv.axon_hooks import get_axon_ntff_profile_hook  # noqa: F401

        return  # real module present
    except ImportError:
        pass
    try:
        import antenv

        mod = types.ModuleType("antenv.axon_hooks")
        _state = {"hook": None}

        def set_axon_ntff_profile_hook(h):
            _state["hook"] = h

        def get_axon_ntff_profile_hook():
            return _state["hook"]

        mod.set_axon_ntff_profile_hook = set_axon_ntff_profile_hook
        mod.get_axon_ntff_profile_hook = get_axon_ntff_profile_hook
        sys.modules["antenv.axon_hooks"] = mod
        antenv.axon_hooks = mod

        from trn_agent_boot.trn_boot import _ntff_profile_via_ctypes

        set_axon_ntff_profile_hook(
            _ntff_profile_via_ctypes("/opt/axon/libaxon_pjrt.so")
        )
    except Exception:
        pass  # profiling degrades; compute still works


_ensure_ntff_hook()

LAST_RESULTS = None
_NC = None


def _build():
    from contextlib import ExitStack

    import concourse.bacc as bacc
    import concourse.mybir as mybir
    import concourse.tile as tile
    from concourse import masks

    f32 = mybir.dt.float32
    f32r = mybir.dt.float32r
    MULT = mybir.AluOpType.mult
    ADD = mybir.AluOpType.add
    EXP = mybir.ActivationFunctionType.Exp

    nc = bacc.Bacc(
        "TRN2", target_bir_lowering=False, debug=False, enable_asserts=False
    )
    ctx_t = nc.dram_tensor("context", (BPC, H, C), f32, kind="ExternalInput").ap()
    q_t = nc.dram_tensor("question", (BPC, H, Q), f32, kind="ExternalInput").ap()
    w_t = nc.dram_tensor("w", (3 * H,), f32, kind="ExternalInput").ap()
    out_t = nc.dram_tensor("out", (BPC, 4 * H, C), f32, kind="ExternalOutput").ap()

    with tile.TileContext(nc) as tc, ExitStack() as ctx:
        const = ctx.enter_context(tc.tile_pool(name="const", bufs=1))
        sb = ctx.enter_context(tc.tile_pool(name="sb", bufs=3))
        ps_score = ctx.enter_context(tc.tile_pool(name="pscore", bufs=2, space="PSUM"))
        ps_tr = ctx.enter_context(tc.tile_pool(name="ptr", bufs=3, space="PSUM"))
        ps_tt = ctx.enter_context(tc.tile_pool(name="ptt", bufs=1, space="PSUM"))

        ident = const.tile([128, 128], f32, tag="ident")
        masks.make_identity(nc, ident[:])
        identr = const.tile([128, 128], f32r, tag="identr")
        nc.vector.tensor_copy(identr[:], ident[:])

        wc = const.tile([128, 1], f32, tag="wc")
        wcq = const.tile([128, 1], f32, tag="wcq")
        nc.sync.dma_start(wc[:], w_t[H : 2 * H].unsqueeze(1))
        nc.sync.dma_start(wcq[:], w_t[2 * H : 3 * H].unsqueeze(1))

        for b in range(BPC):
            X = sb.tile([H, C], f32, tag="X")
            nc.sync.dma_start(X[:], ctx_t[b])
            Y = sb.tile([H, Q], f32, tag="Y")
            nc.sync.dma_start(Y[:], q_t[b])
            # output block 0 is the context itself; emit early to spread DMA
            nc.sync.dma_start(out_t[b, 0:H], X[:])

            # f32r-rounded copies for matmul inputs (halves pipeline better)
            Xr = sb.tile([H, C], f32r, tag="Xr")
            nc.vector.tensor_copy(Xr[:, 0:512], X[:, 0:512])
            nc.vector.tensor_copy(Xr[:, 512:1024], X[:, 512:1024])
            Yr = sb.tile([H, Q], f32r, tag="Yr")
            nc.vector.tensor_copy(Yr[:], Y[:])

            # Z = wcq * Y + wc  (so Z^T @ X = G + 1 cw^T, the softmax logits)
            Z = sb.tile([H, Q], f32r, tag="Z")
            nc.vector.tensor_scalar(Z[:], Y[:], wcq[:], wc[:], op0=MULT, op1=ADD)

            S = ps_score.tile([Q, C], f32, tag="score")
            for j in range(0, C, 512):
                nc.tensor.matmul(
                    S[:, j : j + 512],
                    Z[:],
                    Xr[:, j : j + 512],
                    start=True,
                    stop=True,
                )

            # P = exp(S), d = rowsum(P)  (no max-subtraction: |S| <~ 10)
            P = sb.tile([Q, C], f32r, tag="P")
            dsum = sb.tile([Q, 1], f32, tag="dsum")
            nc.scalar.activation(P[:], S[:], EXP, accum_out=dsum[:])
            rr = sb.tile([Q, 1], f32, tag="rr")
            nc.vector.reciprocal(rr[:], dsum[:])
            r2 = sb.tile([Q, 1], f32, tag="r2")
            nc.vector.tensor_mul(r2[:], rr[:], rr[:])

            # XT holds [YTs | X^T chunks 0..7]; the leading YTs block means
            # every N=256 tt-matmul window reads initialized data.
            XT = sb.tile([128, 128 + C], f32r, tag="XT")

            # YTs = diag(r) Y^T  -> XT[:, 0:128]
            yt = ps_tr.tile([128, 512], f32, tag="tr")
            nc.tensor.transpose(yt[:, 0:128].bitcast(f32r), Yr[:], identr[:])
            YTs = XT[:, 0:128]
            nc.vector.tensor_scalar_mul(YTs, yt[:, 0:128], rr[:])

            # P^T and X^T chunks (PE transposes; copies to SBUF on ScalarE)
            PT = sb.tile([128, C], f32r, tag="PT")
            for g in range(2):
                ptp = ps_tr.tile([128, 512], f32, tag="tr")
                for k in range(4):
                    c0 = g * 4 + k
                    nc.tensor.transpose(
                        ptp[:, k * 128 : (k + 1) * 128].bitcast(f32r),
                        P[:, c0 * 128 : (c0 + 1) * 128],
                        identr[:],
                    )
                nc.scalar.copy(PT[:, g * 512 : (g + 1) * 512], ptp[:])

            for g in range(2):
                xtp = ps_tr.tile([128, 512], f32, tag="tr")
                for k in range(4):
                    c0 = g * 4 + k
                    nc.tensor.transpose(
                        xtp[:, k * 128 : (k + 1) * 128].bitcast(f32r),
                        Xr[:, c0 * 128 : (c0 + 1) * 128],
                        identr[:],
                    )
                nc.scalar.copy(XT[:, 128 + g * 512 : 128 + (g + 1) * 512], xtp[:])

            # tt[:,128:256] = P @ X^T  (cols 0:128 accumulate junk, never read)
            tt = ps_tt.tile([Q, 256], f32, tag="tt")
            for c in range(8):
                nc.tensor.matmul(
                    tt[:],
                    PT[:, c * 128 : (c + 1) * 128],
                    XT[:, c * 128 : c * 128 + 256],
                    start=(c == 0),
                    stop=(c == 7),
                )
            tts = sb.tile([Q, H], f32r, tag="tts")
            nc.vector.tensor_scalar_mul(tts[:], tt[:, 128:256], r2[:])

            # A and B in half-wide PSUM tiles (shared slot pool with the
            # transpose tiles) so score-psum turnover never blocks batch b+1.
            A = sb.tile([H, C], f32, tag="A")
            for j in range(0, C, 512):
                Aps = ps_tr.tile([H, 512], f32, tag="tr")
                nc.tensor.matmul(Aps[:], YTs, P[:, j : j + 512], start=True, stop=True)
                nc.scalar.copy(A[:, j : j + 512], Aps[:])

            Bsb = sb.tile([H, C], f32, tag="B")
            for j in range(0, C, 512):
                Bps = ps_tr.tile([H, 512], f32, tag="tr")
                nc.tensor.matmul(Bps[:], tts[:], P[:, j : j + 512], start=True, stop=True)
                nc.vector.tensor_copy(Bsb[:, j : j + 512], Bps[:])

            XA = sb.tile([H, C], f32, tag="XA")
            nc.vector.tensor_mul(XA[:], X[:], A[:])
            XB = sb.tile([H, C], f32, tag="XB")
            nc.vector.tensor_mul(XB[:], X[:], Bsb[:])

            nc.sync.dma_start(out_t[b, H : 2 * H], A[:])
            nc.sync.dma_start(out_t[b, 2 * H : 3 * H], XA[:])
            nc.sync.dma_start(out_t[b, 3 * H : 4 * H], XB[:])

    nc.compile()
    return nc


def kernel(context, question, w):
    global _NC, LAST_RESULTS
    from concourse import bass_utils

    if _NC is None:
        _NC = _build()

    context = np.ascontiguousarray(np.asarray(context), dtype=np.float32)
    question = np.ascontiguousarray(np.asarray(question), dtype=np.float32)
    w = np.ascontiguousarray(np.asarray(w), dtype=np.float32)

    in_maps = [
        {
            "context": context[c * BPC : (c + 1) * BPC],
            "question": question[c * BPC : (c + 1) * BPC],
            "w": w,
        }
        for c in range(NCORES)
    ]
    trace = bool(int(os.environ.get("KTRACE", "0")))
    LAST_RESULTS = bass_utils.run_bass_kernel_spmd(
        _NC, in_maps, core_ids=list(range(NCORES)), trace=trace
    )
    out = np.concatenate(
        [LAST_RESULTS.results[c]["out"] for c in range(NCORES)], axis=0
    )
    return out
